# revision 15
# baseline (speedup 1.0000x reference)
"""Self-contained Trainium2 Bass kernel: multi-head attention (B=4, N=2048, C=1024, H=16).

Sharding: 8 cores = 4 batches x 2 head-halves (tensor parallel over heads).
Each core computes q/k/v projections for its 8 heads only (512 of 1024
channels), the attention for those heads over its batch's full 2048 rows,
and a PARTIAL output projection (contraction over its 512 channels). The
host sums the two partial [N, C] outputs per batch and adds the bias.
No KV duplication and no on-device collectives.

Attention inner loop processes a quad of heads (2 pairs) per wave, one
512-query chunk at a time, pair ping-pong over the key tiles so the ACT
engine (exp, the critical engine at ~1.09 ns/elem) never idles:
  S^T tiles [128 keys, 2 heads x 512 q] via K=64 row-packed matmul pairs,
  exp -> bf16 P^T in SBUF,
  AV col-packed (tile_position (0,0)/(0,64)) so both heads of a pair share
  the PE array at full width,
  row sums via a ones[128,1] matmul 4-up col-group packed (positions
  (0,0)/(0,32)/(0,64)/(0,96)) accumulating into one shared PSUM bank
  (single start=True clears the bank; the other groups overwrite-where-
  unset on the first key tile).
Softmax skips the max subtraction (|S| < 7 with the 1/8 scale folded into
Wq on the host). Normalization (1/rowsum) is applied to O^T after a
DRAM-broadcast roundtrip, then the partial out-projection is woven into
later waves' PE slack along with the q/k/v projections.

PSUM banks: st_A(2) st_B(2) ot_A(1) ot_B(1) rs(1) proj(1) = 8.
"""

import numpy as np
import ml_dtypes

B, N, C, H = 4, 2048, 1024, 16
DH = C // H                      # 64
SCALE = DH ** -0.5
NCORES = 8
HH = H // 2                      # 8 heads per core (one half)
CH = HH * DH                     # 512 channels per core
NPR = HH // 2                    # 4 head pairs per core
MT = N // 128                    # 16 key tiles
CT = C // 128                    # 8 contraction tiles (input dim)
NQC = N // 512                   # 4 query chunks of 512
PT_DEPTH = 8                     # P^T lookahead slots per pair

_BF16 = ml_dtypes.bfloat16
_cache = {}


def _patch_tile_drain():
    """Walrus in this env rejects >1 sem wait per instruction; split the tail
    Drain's waits into standalone single-wait nops on SP."""
    import concourse.tile as tile
    import concourse.mybir as mybir
    from concourse.vector_clock import ScopedClock

    if getattr(tile.TileContext, "_drain_split_patched", False):
        return

    def _patched(self, tick_clock, wait_clock):
        nc = self.nc
        drain_inst = nc.sync.drain()
        wait_clock.add_sem_waits(
            drain_inst.ins, ScopedClock({None: tick_clock.global_clock})
        )
        si = drain_inst.ins.sync_info
        waits = list(si.on_wait) if si is not None and si.on_wait else []
        if len(waits) > 1:
            si.on_wait = []
            for w in waits:
                nop = nc.sync.nop(hint="drain_wait_split", nofuse=True)
                nsi = nop.ins.sync_info
                if nsi is None:
                    nop.ins.sync_info = mybir.SyncInfo(on_wait=[w], on_update=[])
                else:
                    nsi.on_wait = [w]
        nc.all_engine_barrier()
        assert self.sems is not None
        popped = nc._tile_sem_poison_stack.pop()
        assert popped is self._sem_poison
        nc.clear_and_free_semaphores(list(self.sems.allocated().values()))
        nc.all_engine_barrier()

    tile.TileContext._drain_and_barrier = _patched
    tile.TileContext._drain_split_patched = True


def _split_excess_waits(nc, limit=1):
    """Walrus here rejects instructions carrying more than `limit` sem waits.
    Move the excess onto same-engine nops inserted immediately before."""
    import concourse.mybir as mybir

    counter = [0]
    for block in nc.m.functions[0].blocks:
        il = block.instructions
        i = 0
        while i < len(il):
            inst = il[i]
            si = inst.sync_info
            waits = list(si.on_wait) if si is not None and si.on_wait else []
            if len(waits) > limit:
                keep = waits[-limit:]
                extra = waits[:-limit]
                si.on_wait = keep
                pos = i
                for j in range(0, len(extra), limit):
                    chunk = extra[j : j + limit]
                    counter[0] += 1
                    nop = mybir.InstNoOp(
                        name=f"waitsplit_{counter[0]}",
                        engine=inst.engine,
                        ins=[],
                        outs=[],
                        sync_info=mybir.SyncInfo(on_wait=chunk, on_update=[]),
                    )
                    try:
                        nc.register_instruction(nop, overwrite=True)
                    except Exception:
                        pass
                    il.insert(pos, nop)
                    pos += 1
                    i += 1
            i += 1


def build_nc(debug=False):
    import concourse.bass as bass
    import concourse.mybir as mybir
    import concourse.tile as tile

    _patch_tile_drain()
    f32 = mybir.dt.float32
    bf16 = mybir.dt.bfloat16
    EXP = mybir.ActivationFunctionType.Exp

    nc = bass.Bass("TRN2", num_devices=NCORES)
    xT = nc.dram_tensor("xT", [C, N], bf16, kind="ExternalInput")
    Wq = nc.dram_tensor("Wq", [C, CH], bf16, kind="ExternalInput")
    Wk = nc.dram_tensor("Wk", [C, CH], bf16, kind="ExternalInput")
    Wv = nc.dram_tensor("Wv", [C, CH], bf16, kind="ExternalInput")
    WoT = nc.dram_tensor("WoT", [CH, C], bf16, kind="ExternalInput")
    yT = nc.dram_tensor("yT", [C, N], f32, kind="ExternalOutput")
    rdbg = odbg = None
    if debug:
        rdbg = nc.dram_tensor("rdbg", [HH, N], f32, kind="ExternalOutput")
        odbg = nc.dram_tensor("odbg", [NPR * 128, N], bf16, kind="ExternalOutput")

    with tile.TileContext(nc) as tc:
      with (
          tc.tile_pool(name="persist", bufs=1) as persist,
          tc.tile_pool(name="work", bufs=2) as workp,
          tc.tile_pool(name="dramp", bufs=1, space="DRAM") as dram_pool,
          tc.tile_pool(name="ps_proj", bufs=1, space="PSUM") as ps_proj,
          tc.tile_pool(name="ps_st", bufs=1, space="PSUM") as ps_st,
          tc.tile_pool(name="ps_ot", bufs=1, space="PSUM") as ps_ot,
          tc.tile_pool(name="ps_rs", bufs=1, space="PSUM") as ps_rs,
          tc.tile_pool(name="ptp", bufs=PT_DEPTH) as ptp,
      ):
        # ---------------- persistent SBUF ----------------
        xT_t = persist.tile([128, CT, N], bf16)
        Wq_t = persist.tile([128, CT, CH], bf16)
        Wk_t = persist.tile([128, CT, CH], bf16)
        Wv_t = persist.tile([128, CT, CH], bf16)
        WoT_t = persist.tile([128, NPR, C], bf16)
        qT = [persist.tile([128, N], bf16, name=f"qT{p}") for p in range(NPR)]
        kT = [persist.tile([128, N], bf16, name=f"kT{p}") for p in range(NPR)]
        v_sb = persist.tile([128, MT, CH], bf16)
        OT_fin = [persist.tile([128, N], bf16, name=f"OT{p}") for p in range(NPR)]
        ones_sb = persist.tile([128, 1], bf16)
        rinv_dram = rdbg if debug else dram_pool.tile([HH, N], f32)

        nc.vector.memset(ones_sb, 1.0)

        # ---------------- input DMAs (k/q weights + first x chunk first) ----
        nc.sync.dma_start(out=Wk_t, in_=Wk.ap().rearrange("(a p) d -> p a d", p=128))
        nc.sync.dma_start(out=Wq_t, in_=Wq.ap().rearrange("(a p) d -> p a d", p=128))
        xTr = xT.ap().rearrange("(a p) n -> p a n", p=128)
        for kc in range(NQC):
            ks = slice(kc * 512, (kc + 1) * 512)
            nc.sync.dma_start(out=xT_t[:, :, ks], in_=xTr[:, :, ks])
        nc.sync.dma_start(out=Wv_t, in_=Wv.ap().rearrange("(a p) d -> p a d", p=128))
        nc.sync.dma_start(out=WoT_t, in_=WoT.ap().rearrange("(a p) d -> p a d", p=128))

        # ---------------- weavable projection units ----------------
        # Each unit is (list_of_matmul_closures, finalize_closure). The
        # single proj PSUM bank serializes units via WAR on the DVE copy.
        def kproj_unit(pr, kc, tag="proj"):
            ks = slice(kc * 512, (kc + 1) * 512)
            ps = [None]
            def mk(jc):
                def go():
                    if jc == 0:
                        ps[0] = ps_proj.tile([128, 512], f32, tag=tag, name=f"psk{pr}_{kc}") if tag == "proj" else globals_pool[tag].tile([128, 512], f32, tag=tag, name=f"psk{pr}_{kc}")
                    nc.tensor.matmul(
                        ps[0], Wk_t[:, jc, pr * 128 : (pr + 1) * 128],
                        xT_t[:, jc, ks], start=(jc == 0), stop=(jc == CT - 1),
                    )
                return go
            def fin():
                nc.vector.tensor_copy(out=kT[pr][:, ks], in_=ps[0])
            return [mk(jc) for jc in range(CT)], fin

        def qproj_unit(pr, qc, tag="proj"):
            qs = slice(qc * 512, (qc + 1) * 512)
            ps = [None]
            def mk(jc):
                def go():
                    if jc == 0:
                        ps[0] = ps_proj.tile([128, 512], f32, tag=tag, name=f"psq{pr}_{qc}") if tag == "proj" else globals_pool[tag].tile([128, 512], f32, tag=tag, name=f"psq{pr}_{qc}")
                    nc.tensor.matmul(
                        ps[0], Wq_t[:, jc, pr * 128 : (pr + 1) * 128],
                        xT_t[:, jc, qs], start=(jc == 0), stop=(jc == CT - 1),
                    )
                return go
            def fin():
                nc.vector.tensor_copy(out=qT[pr][:, qs], in_=ps[0])
            return [mk(jc) for jc in range(CT)], fin

        globals_pool = {"ot0": ps_ot, "ot1": ps_ot, "rs": ps_rs}

        def vproj_unit(kt):
            ms = slice(kt * 128, (kt + 1) * 128)
            ps = [None]
            def mk(jc):
                def go():
                    if jc == 0:
                        ps[0] = ps_proj.tile([128, 512], f32, tag="proj", name=f"psv{kt}")
                    nc.tensor.matmul(
                        ps[0], xT_t[:, jc, ms], Wv_t[:, jc, :],
                        start=(jc == 0), stop=(jc == CT - 1),
                    )
                return go
            def fin():
                nc.vector.tensor_copy(out=v_sb[:, kt, :], in_=ps[0])
            return [mk(jc) for jc in range(CT)], fin

        def yproj_unit(ct, qc):
            qs = slice(qc * 512, (qc + 1) * 512)
            ps = [None]
            def mk(pr):
                def go():
                    if pr == 0:
                        ps[0] = ps_proj.tile([128, 512], f32, tag="proj", name=f"psy{ct}_{qc}")
                    nc.tensor.matmul(
                        ps[0], WoT_t[:, pr, ct * 128 : (ct + 1) * 128],
                        OT_fin[pr][:, qs], start=(pr == 0), stop=(pr == NPR - 1),
                    )
                return go
            def fin():
                ys = workp.tile([128, 512], f32, tag="ys", name=f"ys{ct}_{qc}")
                nc.vector.tensor_copy(out=ys, in_=ps[0])
                nc.sync.dma_start(
                    out=yT.ap()[ct * 128 : (ct + 1) * 128, qs], in_=ys
                )
            return [mk(pr) for pr in range(NPR)], fin

        # weave pump state: a flat list of pending closures
        weave = []          # list of (key, closure)
        unit_end = {}       # key -> index of its last closure in `weave`
        wv_pos = [0]

        def push_unit(key, unit):
            mms, fin = unit
            for m in mms:
                weave.append((key, m))
            weave.append((key, fin))
            unit_end[key] = len(weave) - 1

        def pump(n):
            """Emit up to n pending weave closures."""
            e = 0
            while e < n and wv_pos[0] < len(weave):
                weave[wv_pos[0]][1]()
                wv_pos[0] += 1
                e += 1

        def pump_until(key):
            """Emit weave closures until unit `key` is fully out (no-op if
            already emitted)."""
            idx = unit_end.get(key, -1)
            while wv_pos[0] <= idx:
                weave[wv_pos[0]][1]()
                wv_pos[0] += 1

        def unit_done(key):
            return unit_end.get(key, 10 ** 9) < wv_pos[0]

        # ---------------- startup projections ----------------
        # spread the first four units over idle psum banks (ot/rs unused
        # until the first wave's AV) to avoid single-bank serialization
        for u in (
            ("k00", kproj_unit(0, 0, tag="proj")),
            ("k10", kproj_unit(1, 0, tag="ot0")),
            ("q00", qproj_unit(0, 0, tag="ot1")),
            ("q10", qproj_unit(1, 0, tag="rs")),
        ):
            push_unit(*u)
        pump_until("q10")

        # weave order: wave-0 (quad A, qc0) needs kproj 0/1 by kt=4*kc and
        # vproj progressively (AV is v-paced); wave-1 (A, qc1) needs q01/q11;
        # wave-2 (B, qc0) prep follows. Later qproj/yproj are appended
        # dynamically at wave boundaries.
        push_unit("v0", vproj_unit(0)); push_unit("v1", vproj_unit(1))
        push_unit("k01", kproj_unit(0, 1)); push_unit("k11", kproj_unit(1, 1))
        push_unit("v2", vproj_unit(2)); push_unit("v3", vproj_unit(3))
        push_unit("k02", kproj_unit(0, 2)); push_unit("k12", kproj_unit(1, 2))
        push_unit("v4", vproj_unit(4)); push_unit("v5", vproj_unit(5))
        push_unit("k03", kproj_unit(0, 3)); push_unit("k13", kproj_unit(1, 3))
        for kt in range(6, MT):
            push_unit(f"v{kt}", vproj_unit(kt))
        push_unit("q01", qproj_unit(0, 1)); push_unit("q11", qproj_unit(1, 1))
        push_unit("k20", kproj_unit(2, 0)); push_unit("k30", kproj_unit(3, 0))
        push_unit("q20", qproj_unit(2, 0)); push_unit("q30", qproj_unit(3, 0))
        for kc in range(1, NQC):
            push_unit(f"k2{kc}", kproj_unit(2, kc))
            push_unit(f"k3{kc}", kproj_unit(3, kc))

        # ---------------- attention waves ----------------
        # quad A = pairs 0,1 (heads 0-3); quad B = pairs 2,3 (heads 4-7).
        # A's q-chunks lead by one wave so B's projections have pump slack.
        waves = [(0, 0), (0, 1), (1, 0), (1, 1), (0, 2), (1, 2), (0, 3), (1, 3)]
        first_wave_of_quad = {0: 0, 1: 2}

        def v_ready(kt):
            return unit_done(f"v{kt}")

        for wi, (quad, qc) in enumerate(waves):
            p0, p1 = 2 * quad, 2 * quad + 1
            qs = slice(qc * 512, (qc + 1) * 512)
            first_wave = wi == 0

            # prerequisites: qT for this quad+chunk, kT chunk 0
            if wi > 0:
                pump_until(f"q{p1}{qc}")

            st = {}
            pt = {}
            ot = {}

            def S_pair(pslot, pr, kt):
                ms = slice(kt * 128, (kt + 1) * 128)
                t = ps_st.tile([128, 1024], f32, tag=f"st{pslot}", name=f"st{pslot}_{kt}")
                nc.tensor.matmul(
                    t[:, 0:512], kT[pr][0:64, ms], qT[pr][0:64, qs],
                    start=True, stop=True, tile_position=(0, 0),
                )
                nc.tensor.matmul(
                    t[:, 512:1024], kT[pr][64:128, ms], qT[pr][64:128, qs],
                    start=True, stop=True, tile_position=(64, 0),
                )
                st[pslot] = t

            def EXP_pair(pslot, kt):
                t = ptp.tile([128, 1024], bf16, tag=f"pt{pslot}", name=f"pt{pslot}_{kt}")
                nc.scalar.activation(out=t, in_=st[pslot], func=EXP)
                pt[(pslot, kt)] = t

            def AV_pair(pslot, pr, kt):
                he, ho = (2 * pr) * DH, (2 * pr + 1) * DH
                p = pt.pop((pslot, kt))
                nc.tensor.matmul(
                    ot[pslot][0:64, :], v_sb[:, kt, he : he + DH], p[:, 0:512],
                    start=(kt == 0), stop=(kt == MT - 1),
                    tile_position=(0, 0),
                )
                nc.tensor.matmul(
                    ot[pslot][64:128, :], v_sb[:, kt, ho : ho + DH],
                    p[:, 512:1024],
                    start=(kt == 0), stop=(kt == MT - 1),
                    tile_position=(0, 64), skip_group_check=(kt == 0),
                )
                return p

            def RS(kt, pts):
                # 4-up col-group packed ones-matmul; one shared PSUM bank.
                for j in range(4):
                    p = pts[j // 2]
                    nc.tensor.matmul(
                        rs_ps[32 * j : 32 * j + 1, :],
                        ones_sb, p[:, (j % 2) * 512 : (j % 2) * 512 + 512],
                        start=(kt == 0), stop=(kt == MT - 1),
                        tile_position=(0, 32 * j),
                        skip_group_check=(j != 0),
                    )

            ot[0] = ps_ot.tile([128, 512], f32, tag="ot0", name=f"ot0_{wi}")
            ot[1] = ps_ot.tile([128, 512], f32, tag="ot1", name=f"ot1_{wi}")
            rs_ps = ps_rs.tile([128, 512], f32, tag="rs", name=f"rs_{wi}")

            av_kt = [0]   # next kt whose AV/RS is pending emission
            pend = {}     # kt -> (p_tile0, p_tile1) kept alive until RS

            def drain_av(upto):
                while av_kt[0] <= upto:
                    j = av_kt[0]
                    if first_wave and not v_ready(j):
                        break
                    pa = AV_pair(0, p0, j)
                    pb = AV_pair(1, p1, j)
                    RS(j, (pa, pb))
                    av_kt[0] += 1

            for kt in range(MT):
                if wi == first_wave_of_quad[quad] and kt in (4, 8, 12):
                    # kproj deadlines are hard: S(kt) needs kT chunk kt//4
                    pump_until(f"k{p0}{kt // 4}")
                    pump_until(f"k{p1}{kt // 4}")
                S_pair(0, p0, kt)
                EXP_pair(0, kt)
                S_pair(1, p1, kt)
                EXP_pair(1, kt)
                # lookahead guard: don't let exp run more than PT_DEPTH-1
                # ahead of AV consumption
                if kt - av_kt[0] >= PT_DEPTH - 1:
                    if first_wave:
                        pump_until(f"v{av_kt[0]}")
                    drain_av(av_kt[0])
                drain_av(kt - 2)
                pump(8 if first_wave else 5)
            if first_wave:
                for j in range(av_kt[0], MT):
                    pump_until(f"v{j}")
                    drain_av(j)
            else:
                drain_av(MT - 1)

            # ---------------- wave drain: free psum fast ----------------
            otu0 = workp.tile([128, 512], bf16, tag="otu0", name=f"otu0_{wi}")
            otu1 = workp.tile([128, 512], bf16, tag="otu1", name=f"otu1_{wi}")
            nc.vector.tensor_copy(out=otu0, in_=ot[0])
            nc.vector.tensor_copy(out=otu1, in_=ot[1])
            rinv_st = workp.tile([128, 512], f32, tag="rinv", name=f"rinv_{wi}")
            nc.vector.reciprocal(out=rinv_st, in_=rs_ps)
            for j in range(4):
                nc.sync.dma_start(
                    out=rinv_dram[quad * 4 + j : quad * 4 + j + 1, qs],
                    in_=rinv_st[32 * j : 32 * j + 1, :],
                )
            for i, (pr, otu) in enumerate(((p0, otu0), (p1, otu1))):
                rbc = workp.tile([128, 512], f32, tag=f"rbc{i}", name=f"rbc{i}_{wi}")
                re, ro = quad * 4 + 2 * i, quad * 4 + 2 * i + 1
                nc.sync.dma_start(
                    out=rbc[0:64, :],
                    in_=rinv_dram[re : re + 1, qs].to_broadcast([64, 512]),
                )
                nc.sync.dma_start(
                    out=rbc[64:128, :],
                    in_=rinv_dram[ro : ro + 1, qs].to_broadcast([64, 512]),
                )
                nc.vector.tensor_mul(OT_fin[pr][:, qs], otu, rbc)

            # schedule follow-on weave work (skip q-units already queued
            # statically for waves 0-2)
            if wi + 2 < len(waves):
                nquad, nqc = waves[wi + 2]
                for npr in (2 * nquad, 2 * nquad + 1):
                    if f"q{npr}{nqc}" not in unit_end:
                        push_unit(f"q{npr}{nqc}", qproj_unit(npr, nqc))
            if quad == 1:
                for ct in range(CT):
                    push_unit(f"y{ct}_{qc}", yproj_unit(ct, qc))

        # drain any remaining weave work (last q-chunk's out-projection)
        pump(10 ** 9)
        if debug:
            for pr in range(NPR):
                nc.sync.dma_start(
                    out=odbg[pr * 128 : (pr + 1) * 128, :], in_=OT_fin[pr]
                )

    _split_excess_waits(nc)
    return nc


def make_in_maps(x, Wq, Wkv, Wout, bout):
    x = np.asarray(x, dtype=np.float32)
    Wq = np.asarray(Wq, dtype=np.float32)
    Wkv = np.asarray(Wkv, dtype=np.float32)
    Wout = np.asarray(Wout, dtype=np.float32)
    in_maps = []
    for core in range(NCORES):
        b, g = core // 2, core % 2
        cs = slice(g * CH, (g + 1) * CH)
        in_maps.append(
            dict(
                xT=np.ascontiguousarray(x[b].T).astype(_BF16),
                Wq=np.ascontiguousarray(Wq[:, cs] * SCALE).astype(_BF16),
                Wk=np.ascontiguousarray(Wkv[:, cs]).astype(_BF16),
                Wv=np.ascontiguousarray(Wkv[:, C:][:, cs]).astype(_BF16),
                WoT=np.ascontiguousarray(Wout[cs, :]).astype(_BF16),
            )
        )
    return in_maps


def assemble(results, bout=None):
    out = np.empty((B, N, C), dtype=np.float32)
    for b in range(B):
        acc = results[2 * b]["yT"].T + results[2 * b + 1]["yT"].T
        out[b] = acc
    if bout is not None:
        out += np.asarray(bout, dtype=np.float32)
    return out


def kernel(x, Wq, Wkv, Wout, bout):
    from concourse.bass_utils import run_bass_kernel_spmd

    if "nc" not in _cache:
        _cache["nc"] = build_nc()
    in_maps = make_in_maps(x, Wq, Wkv, Wout, bout)
    res = run_bass_kernel_spmd(_cache["nc"], in_maps, core_ids=list(range(NCORES)))
    return assemble(res.results, bout)


# revision 16
# speedup vs baseline: 1.0244x; 1.0244x over previous
"""Self-contained Trainium2 Bass kernel: multi-head attention (B=4, N=2048, C=1024, H=16).

Sharding: 8 cores = 4 batches x 2 head-halves (tensor parallel over heads).
Each core computes q/k/v projections for its 8 heads only (512 of 1024
channels), the attention for those heads over its batch's full 2048 rows,
and a PARTIAL output projection (contraction over its 512 channels). The
host sums the two partial [N, C] outputs per batch and adds the bias.
No KV duplication and no on-device collectives.

Attention inner loop processes a quad of heads (2 pairs) per wave, one
512-query chunk at a time, pair ping-pong over the key tiles so the ACT
engine (exp, the critical engine at ~1.09 ns/elem) never idles:
  S^T tiles [128 keys, 2 heads x 512 q] via K=64 row-packed matmul pairs,
  exp -> bf16 P^T in SBUF,
  AV col-packed (tile_position (0,0)/(0,64)) so both heads of a pair share
  the PE array at full width,
  row sums via a ones[128,1] matmul 4-up col-group packed (positions
  (0,0)/(0,32)/(0,64)/(0,96)) accumulating into one shared PSUM bank
  (single start=True clears the bank; the other groups overwrite-where-
  unset on the first key tile).
Softmax skips the max subtraction (|S| < 7 with the 1/8 scale folded into
Wq on the host). Normalization (1/rowsum) is applied to O^T after a
DRAM-broadcast roundtrip, then the partial out-projection is woven into
later waves' PE slack along with the q/k/v projections.

PSUM banks: st_A(2) st_B(2) ot_A(1) ot_B(1) rs(1) proj(1) = 8.
"""

import numpy as np
import ml_dtypes

B, N, C, H = 4, 2048, 1024, 16
DH = C // H                      # 64
SCALE = DH ** -0.5
NCORES = 8
HH = H // 2                      # 8 heads per core (one half)
CH = HH * DH                     # 512 channels per core
NPR = HH // 2                    # 4 head pairs per core
MT = N // 128                    # 16 key tiles
CT = C // 128                    # 8 contraction tiles (input dim)
NQC = N // 512                   # 4 query chunks of 512
PT_DEPTH = 10                    # P^T lookahead slots per pair

_BF16 = ml_dtypes.bfloat16
_cache = {}


def _patch_tile_drain():
    """Walrus in this env rejects >1 sem wait per instruction; split the tail
    Drain's waits into standalone single-wait nops on SP."""
    import concourse.tile as tile
    import concourse.mybir as mybir
    from concourse.vector_clock import ScopedClock

    if getattr(tile.TileContext, "_drain_split_patched", False):
        return

    def _patched(self, tick_clock, wait_clock):
        nc = self.nc
        drain_inst = nc.sync.drain()
        wait_clock.add_sem_waits(
            drain_inst.ins, ScopedClock({None: tick_clock.global_clock})
        )
        si = drain_inst.ins.sync_info
        waits = list(si.on_wait) if si is not None and si.on_wait else []
        if len(waits) > 1:
            si.on_wait = []
            for w in waits:
                nop = nc.sync.nop(hint="drain_wait_split", nofuse=True)
                nsi = nop.ins.sync_info
                if nsi is None:
                    nop.ins.sync_info = mybir.SyncInfo(on_wait=[w], on_update=[])
                else:
                    nsi.on_wait = [w]
        nc.all_engine_barrier()
        assert self.sems is not None
        popped = nc._tile_sem_poison_stack.pop()
        assert popped is self._sem_poison
        nc.clear_and_free_semaphores(list(self.sems.allocated().values()))
        nc.all_engine_barrier()

    tile.TileContext._drain_and_barrier = _patched
    tile.TileContext._drain_split_patched = True


def _split_excess_waits(nc, limit=1):
    """Walrus here rejects instructions carrying more than `limit` sem waits.
    Move the excess onto same-engine nops inserted immediately before."""
    import concourse.mybir as mybir

    counter = [0]
    for block in nc.m.functions[0].blocks:
        il = block.instructions
        i = 0
        while i < len(il):
            inst = il[i]
            si = inst.sync_info
            waits = list(si.on_wait) if si is not None and si.on_wait else []
            if len(waits) > limit:
                keep = waits[-limit:]
                extra = waits[:-limit]
                si.on_wait = keep
                pos = i
                for j in range(0, len(extra), limit):
                    chunk = extra[j : j + limit]
                    counter[0] += 1
                    nop = mybir.InstNoOp(
                        name=f"waitsplit_{counter[0]}",
                        engine=inst.engine,
                        ins=[],
                        outs=[],
                        sync_info=mybir.SyncInfo(on_wait=chunk, on_update=[]),
                    )
                    try:
                        nc.register_instruction(nop, overwrite=True)
                    except Exception:
                        pass
                    il.insert(pos, nop)
                    pos += 1
                    i += 1
            i += 1


def build_nc(debug=False):
    import concourse.bass as bass
    import concourse.mybir as mybir
    import concourse.tile as tile

    _patch_tile_drain()
    f32 = mybir.dt.float32
    bf16 = mybir.dt.bfloat16
    EXP = mybir.ActivationFunctionType.Exp

    nc = bass.Bass("TRN2", num_devices=NCORES)
    xT = nc.dram_tensor("xT", [C, N], bf16, kind="ExternalInput")
    Wq = nc.dram_tensor("Wq", [C, CH], bf16, kind="ExternalInput")
    Wk = nc.dram_tensor("Wk", [C, CH], bf16, kind="ExternalInput")
    Wv = nc.dram_tensor("Wv", [C, CH], bf16, kind="ExternalInput")
    WoT = nc.dram_tensor("WoT", [CH, C], bf16, kind="ExternalInput")
    yT = nc.dram_tensor("yT", [C, N], f32, kind="ExternalOutput")
    rdbg = odbg = None
    if debug:
        rdbg = nc.dram_tensor("rdbg", [HH, N], f32, kind="ExternalOutput")
        odbg = nc.dram_tensor("odbg", [NPR * 128, N], bf16, kind="ExternalOutput")

    with tile.TileContext(nc) as tc:
      with (
          tc.tile_pool(name="persist", bufs=1) as persist,
          tc.tile_pool(name="work", bufs=2) as workp,
          tc.tile_pool(name="dramp", bufs=1, space="DRAM") as dram_pool,
          tc.tile_pool(name="ps_proj", bufs=1, space="PSUM") as ps_proj,
          tc.tile_pool(name="ps_st", bufs=1, space="PSUM") as ps_st,
          tc.tile_pool(name="ps_ot", bufs=1, space="PSUM") as ps_ot,
          tc.tile_pool(name="ps_rs", bufs=1, space="PSUM") as ps_rs,
          tc.tile_pool(name="ptp", bufs=PT_DEPTH) as ptp,
      ):
        # ---------------- persistent SBUF ----------------
        xT_t = persist.tile([128, CT, N], bf16)
        Wq_t = persist.tile([128, CT, CH], bf16)
        Wk_t = persist.tile([128, CT, CH], bf16)
        Wv_t = persist.tile([128, CT, CH], bf16)
        WoT_t = persist.tile([128, NPR, C], bf16)
        qT = [persist.tile([128, N], bf16, name=f"qT{p}") for p in range(NPR)]
        kT = [persist.tile([128, N], bf16, name=f"kT{p}") for p in range(NPR)]
        v_sb = persist.tile([128, MT, CH], bf16)
        OT_fin = [persist.tile([128, N], bf16, name=f"OT{p}") for p in range(NPR)]
        ones_sb = persist.tile([128, 1], bf16)
        rinv_dram = rdbg if debug else dram_pool.tile([HH, N], f32)

        nc.vector.memset(ones_sb, 1.0)

        # ---------------- input DMAs (k/q weights + first x chunk first) ----
        nc.sync.dma_start(out=Wk_t, in_=Wk.ap().rearrange("(a p) d -> p a d", p=128))
        nc.sync.dma_start(out=Wq_t, in_=Wq.ap().rearrange("(a p) d -> p a d", p=128))
        xTr = xT.ap().rearrange("(a p) n -> p a n", p=128)
        for kc in range(NQC):
            ks = slice(kc * 512, (kc + 1) * 512)
            nc.sync.dma_start(out=xT_t[:, :, ks], in_=xTr[:, :, ks])
        nc.sync.dma_start(out=Wv_t, in_=Wv.ap().rearrange("(a p) d -> p a d", p=128))
        nc.sync.dma_start(out=WoT_t, in_=WoT.ap().rearrange("(a p) d -> p a d", p=128))

        # ---------------- weavable projection units ----------------
        # Each unit is (list_of_matmul_closures, finalize_closure). The
        # single proj PSUM bank serializes units via WAR on the DVE copy.
        def kproj_unit(pr, kc, tag="proj"):
            ks = slice(kc * 512, (kc + 1) * 512)
            ps = [None]
            def mk(jc):
                def go():
                    if jc == 0:
                        ps[0] = ps_proj.tile([128, 512], f32, tag=tag, name=f"psk{pr}_{kc}") if tag == "proj" else globals_pool[tag].tile([128, 512], f32, tag=tag, name=f"psk{pr}_{kc}")
                    nc.tensor.matmul(
                        ps[0], Wk_t[:, jc, pr * 128 : (pr + 1) * 128],
                        xT_t[:, jc, ks], start=(jc == 0), stop=(jc == CT - 1),
                    )
                return go
            def fin():
                nc.vector.tensor_copy(out=kT[pr][:, ks], in_=ps[0])
            return [mk(jc) for jc in range(CT)], fin

        def qproj_unit(pr, qc, tag="proj"):
            qs = slice(qc * 512, (qc + 1) * 512)
            ps = [None]
            def mk(jc):
                def go():
                    if jc == 0:
                        ps[0] = ps_proj.tile([128, 512], f32, tag=tag, name=f"psq{pr}_{qc}") if tag == "proj" else globals_pool[tag].tile([128, 512], f32, tag=tag, name=f"psq{pr}_{qc}")
                    nc.tensor.matmul(
                        ps[0], Wq_t[:, jc, pr * 128 : (pr + 1) * 128],
                        xT_t[:, jc, qs], start=(jc == 0), stop=(jc == CT - 1),
                    )
                return go
            def fin():
                nc.vector.tensor_copy(out=qT[pr][:, qs], in_=ps[0])
            return [mk(jc) for jc in range(CT)], fin

        globals_pool = {"ot0": ps_ot, "ot1": ps_ot, "rs": ps_rs}

        def vproj_unit(kt, tag="proj"):
            ms = slice(kt * 128, (kt + 1) * 128)
            ps = [None]
            def mk(jc):
                def go():
                    if jc == 0:
                        pool = ps_proj if tag == "proj" else globals_pool[tag]
                        ps[0] = pool.tile([128, 512], f32, tag=tag, name=f"psv{kt}")
                    nc.tensor.matmul(
                        ps[0], xT_t[:, jc, ms], Wv_t[:, jc, :],
                        start=(jc == 0), stop=(jc == CT - 1),
                    )
                return go
            def fin():
                nc.vector.tensor_copy(out=v_sb[:, kt, :], in_=ps[0])
            return [mk(jc) for jc in range(CT)], fin

        def yproj_unit(ct, qc):
            qs = slice(qc * 512, (qc + 1) * 512)
            ps = [None]
            def mk(pr):
                def go():
                    if pr == 0:
                        ps[0] = ps_proj.tile([128, 512], f32, tag="proj", name=f"psy{ct}_{qc}")
                    nc.tensor.matmul(
                        ps[0], WoT_t[:, pr, ct * 128 : (ct + 1) * 128],
                        OT_fin[pr][:, qs], start=(pr == 0), stop=(pr == NPR - 1),
                    )
                return go
            def fin():
                ys = workp.tile([128, 512], f32, tag="ys", name=f"ys{ct}_{qc}")
                nc.vector.tensor_copy(out=ys, in_=ps[0])
                nc.sync.dma_start(
                    out=yT.ap()[ct * 128 : (ct + 1) * 128, qs], in_=ys
                )
            return [mk(pr) for pr in range(NPR)], fin

        # weave pump state: a flat list of pending closures
        weave = []          # list of (key, closure)
        unit_end = {}       # key -> index of its last closure in `weave`
        wv_pos = [0]

        def push_unit(key, unit):
            mms, fin = unit
            for m in mms:
                weave.append((key, m))
            weave.append((key, fin))
            unit_end[key] = len(weave) - 1

        def pump(n):
            """Emit up to n pending weave closures."""
            e = 0
            while e < n and wv_pos[0] < len(weave):
                weave[wv_pos[0]][1]()
                wv_pos[0] += 1
                e += 1

        def pump_until(key):
            """Emit weave closures until unit `key` is fully out (no-op if
            already emitted)."""
            idx = unit_end.get(key, -1)
            while wv_pos[0] <= idx:
                weave[wv_pos[0]][1]()
                wv_pos[0] += 1

        def unit_done(key):
            return unit_end.get(key, 10 ** 9) < wv_pos[0]

        # ---------------- startup projections ----------------
        # spread startup units over idle psum banks (ot/rs unused until the
        # first wave's AV) to avoid single-bank serialization; v0..v7 are
        # front-loaded here so wave 0's weave only carries v8..v15 + kproj.
        tags4 = ["proj", "ot0", "ot1", "rs"]
        start_units = [
            ("k00", kproj_unit(0, 0, tag="proj")),
            ("k10", kproj_unit(1, 0, tag="ot0")),
            ("q00", qproj_unit(0, 0, tag="ot1")),
            ("q10", qproj_unit(1, 0, tag="rs")),
        ]
        for i in range(8):
            start_units.append((f"v{i}", vproj_unit(i, tag=tags4[i % 4])))
        for u in start_units:
            push_unit(*u)
        pump_until("v7")

        # weave order: wave-0 (quad A, qc0) needs kproj 0/1 by kt=4*kc and
        # v8..15 for its (deferred) AV; wave-1 (A, qc1) needs q01/q11 at
        # entry; wave-2 (B, qc0) prep follows. Later qproj/yproj are
        # appended dynamically at wave boundaries.
        push_unit("k01", kproj_unit(0, 1)); push_unit("k11", kproj_unit(1, 1))
        push_unit("v8", vproj_unit(8)); push_unit("v9", vproj_unit(9))
        push_unit("k02", kproj_unit(0, 2)); push_unit("k12", kproj_unit(1, 2))
        push_unit("v10", vproj_unit(10)); push_unit("v11", vproj_unit(11))
        push_unit("k03", kproj_unit(0, 3)); push_unit("k13", kproj_unit(1, 3))
        for kt in range(12, MT):
            push_unit(f"v{kt}", vproj_unit(kt))
        push_unit("q01", qproj_unit(0, 1)); push_unit("q11", qproj_unit(1, 1))
        push_unit("k20", kproj_unit(2, 0)); push_unit("k30", kproj_unit(3, 0))
        push_unit("q20", qproj_unit(2, 0)); push_unit("q30", qproj_unit(3, 0))
        for kc in range(1, NQC):
            push_unit(f"k2{kc}", kproj_unit(2, kc))
            push_unit(f"k3{kc}", kproj_unit(3, kc))

        # ---------------- attention waves ----------------
        # quad A = pairs 0,1 (heads 0-3); quad B = pairs 2,3 (heads 4-7).
        # A's q-chunks lead by one wave so B's projections have pump slack.
        waves = [(0, 0), (0, 1), (1, 0), (1, 1), (0, 2), (1, 2), (0, 3), (1, 3)]
        first_wave_of_quad = {0: 0, 1: 2}

        def v_ready(kt):
            return unit_done(f"v{kt}")

        for wi, (quad, qc) in enumerate(waves):
            p0, p1 = 2 * quad, 2 * quad + 1
            qs = slice(qc * 512, (qc + 1) * 512)
            first_wave = wi == 0

            # prerequisites: qT for this quad+chunk, kT chunk 0
            if wi > 0:
                pump_until(f"q{p1}{qc}")

            st = {}
            pt = {}
            ot = {}

            def S_pair(pslot, pr, kt):
                ms = slice(kt * 128, (kt + 1) * 128)
                t = ps_st.tile([128, 1024], f32, tag=f"st{pslot}", name=f"st{pslot}_{kt}")
                nc.tensor.matmul(
                    t[:, 0:512], kT[pr][0:64, ms], qT[pr][0:64, qs],
                    start=True, stop=True, tile_position=(0, 0),
                )
                nc.tensor.matmul(
                    t[:, 512:1024], kT[pr][64:128, ms], qT[pr][64:128, qs],
                    start=True, stop=True, tile_position=(64, 0),
                )
                st[pslot] = t

            def EXP_pair(pslot, kt):
                t = ptp.tile([128, 1024], bf16, tag=f"pt{pslot}", name=f"pt{pslot}_{kt}")
                nc.scalar.activation(out=t, in_=st[pslot], func=EXP)
                pt[(pslot, kt)] = t

            def AV_pair(pslot, pr, kt):
                he, ho = (2 * pr) * DH, (2 * pr + 1) * DH
                p = pt.pop((pslot, kt))
                nc.tensor.matmul(
                    ot[pslot][0:64, :], v_sb[:, kt, he : he + DH], p[:, 0:512],
                    start=(kt == 0), stop=(kt == MT - 1),
                    tile_position=(0, 0),
                )
                nc.tensor.matmul(
                    ot[pslot][64:128, :], v_sb[:, kt, ho : ho + DH],
                    p[:, 512:1024],
                    start=(kt == 0), stop=(kt == MT - 1),
                    tile_position=(0, 64), skip_group_check=(kt == 0),
                )
                return p

            def RS(kt, pts):
                # 4-up col-group packed ones-matmul; one shared PSUM bank.
                for j in range(4):
                    p = pts[j // 2]
                    nc.tensor.matmul(
                        rs_ps[32 * j : 32 * j + 1, :],
                        ones_sb, p[:, (j % 2) * 512 : (j % 2) * 512 + 512],
                        start=(kt == 0), stop=(kt == MT - 1),
                        tile_position=(0, 32 * j),
                        skip_group_check=(j != 0),
                    )

            ot[0] = ps_ot.tile([128, 512], f32, tag="ot0", name=f"ot0_{wi}")
            ot[1] = ps_ot.tile([128, 512], f32, tag="ot1", name=f"ot1_{wi}")
            rs_ps = ps_rs.tile([128, 512], f32, tag="rs", name=f"rs_{wi}")

            av_kt = [0]   # next kt whose AV/RS is pending emission
            pend = {}     # kt -> (p_tile0, p_tile1) kept alive until RS

            def drain_av(upto):
                while av_kt[0] <= upto:
                    j = av_kt[0]
                    if first_wave and not v_ready(j):
                        break
                    pa = AV_pair(0, p0, j)
                    pb = AV_pair(1, p1, j)
                    RS(j, (pa, pb))
                    av_kt[0] += 1

            for kt in range(MT):
                if wi == first_wave_of_quad[quad] and kt in (4, 8, 12):
                    # kproj deadlines are hard: S(kt) needs kT chunk kt//4
                    pump_until(f"k{p0}{kt // 4}")
                    pump_until(f"k{p1}{kt // 4}")
                S_pair(0, p0, kt)
                EXP_pair(0, kt)
                S_pair(1, p1, kt)
                EXP_pair(1, kt)
                # lookahead guard: don't let exp run more than PT_DEPTH-1
                # ahead of AV consumption
                if kt - av_kt[0] >= PT_DEPTH - 1:
                    if first_wave:
                        pump_until(f"v{av_kt[0]}")
                    drain_av(av_kt[0])
                drain_av(kt - 2)
                pump(8 if wi <= 1 else 5)
            if first_wave:
                for j in range(av_kt[0], MT):
                    pump_until(f"v{j}")
                    drain_av(j)
            else:
                drain_av(MT - 1)

            # ---------------- wave drain: free psum fast ----------------
            otu0 = workp.tile([128, 512], bf16, tag="otu0", name=f"otu0_{wi}")
            otu1 = workp.tile([128, 512], bf16, tag="otu1", name=f"otu1_{wi}")
            nc.vector.tensor_copy(out=otu0, in_=ot[0])
            nc.vector.tensor_copy(out=otu1, in_=ot[1])
            rinv_st = workp.tile([128, 512], f32, tag="rinv", name=f"rinv_{wi}")
            nc.vector.reciprocal(out=rinv_st, in_=rs_ps)
            for j in range(4):
                nc.sync.dma_start(
                    out=rinv_dram[quad * 4 + j : quad * 4 + j + 1, qs],
                    in_=rinv_st[32 * j : 32 * j + 1, :],
                )
            for i, (pr, otu) in enumerate(((p0, otu0), (p1, otu1))):
                rbc = workp.tile([128, 512], f32, tag=f"rbc{i}", name=f"rbc{i}_{wi}")
                re, ro = quad * 4 + 2 * i, quad * 4 + 2 * i + 1
                nc.sync.dma_start(
                    out=rbc[0:64, :],
                    in_=rinv_dram[re : re + 1, qs].to_broadcast([64, 512]),
                )
                nc.sync.dma_start(
                    out=rbc[64:128, :],
                    in_=rinv_dram[ro : ro + 1, qs].to_broadcast([64, 512]),
                )
                nc.vector.tensor_mul(OT_fin[pr][:, qs], otu, rbc)

            # schedule follow-on weave work (skip q-units already queued
            # statically for waves 0-2)
            if wi + 2 < len(waves):
                nquad, nqc = waves[wi + 2]
                for npr in (2 * nquad, 2 * nquad + 1):
                    if f"q{npr}{nqc}" not in unit_end:
                        push_unit(f"q{npr}{nqc}", qproj_unit(npr, nqc))
            if quad == 1:
                for ct in range(CT):
                    push_unit(f"y{ct}_{qc}", yproj_unit(ct, qc))

        # drain any remaining weave work (last q-chunk's out-projection)
        pump(10 ** 9)
        if debug:
            for pr in range(NPR):
                nc.sync.dma_start(
                    out=odbg[pr * 128 : (pr + 1) * 128, :], in_=OT_fin[pr]
                )

    _split_excess_waits(nc)
    return nc


def make_in_maps(x, Wq, Wkv, Wout, bout):
    x = np.asarray(x, dtype=np.float32)
    Wq = np.asarray(Wq, dtype=np.float32)
    Wkv = np.asarray(Wkv, dtype=np.float32)
    Wout = np.asarray(Wout, dtype=np.float32)
    in_maps = []
    for core in range(NCORES):
        b, g = core // 2, core % 2
        cs = slice(g * CH, (g + 1) * CH)
        in_maps.append(
            dict(
                xT=np.ascontiguousarray(x[b].T).astype(_BF16),
                Wq=np.ascontiguousarray(Wq[:, cs] * SCALE).astype(_BF16),
                Wk=np.ascontiguousarray(Wkv[:, cs]).astype(_BF16),
                Wv=np.ascontiguousarray(Wkv[:, C:][:, cs]).astype(_BF16),
                WoT=np.ascontiguousarray(Wout[cs, :]).astype(_BF16),
            )
        )
    return in_maps


def assemble(results, bout=None):
    out = np.empty((B, N, C), dtype=np.float32)
    for b in range(B):
        acc = results[2 * b]["yT"].T + results[2 * b + 1]["yT"].T
        out[b] = acc
    if bout is not None:
        out += np.asarray(bout, dtype=np.float32)
    return out


def kernel(x, Wq, Wkv, Wout, bout):
    from concourse.bass_utils import run_bass_kernel_spmd

    if "nc" not in _cache:
        _cache["nc"] = build_nc()
    in_maps = make_in_maps(x, Wq, Wkv, Wout, bout)
    res = run_bass_kernel_spmd(_cache["nc"], in_maps, core_ids=list(range(NCORES)))
    return assemble(res.results, bout)
